# revision 3
# baseline (speedup 1.0000x reference)
"""CondConv2d Bass kernel for 8 Trainium2 NeuronCores.

Reference semantics (per sample b):
    pooled  = mean(x[b], axis=(H,W))                      # [C]
    r       = sigmoid(fc_w @ pooled + fc_b)               # [E]
    W_eff   = sum_e r[e] * weight[e]                      # [O, I, 3, 3]
    out[b]  = conv2d(x[b], W_eff, stride=1, pad=1)        # [O, H, W]

Sharding: data-parallel over batch B=32 across 8 cores (4 samples/core);
weights + routing FC replicated.

Per-core kernel design:
 - x image zero-padded to 58x58 in SBUF (one [128, 2, 58, 58] f32r tile
   per double-buffer slot); borders stay zero so a full-plane reduce
   equals the unpadded spatial sum.
 - pooling: ACT activation(Copy) with accum_out (in-place, scale=1);
   the 1/(H*W) mean factor is folded into the fc_w constant.
 - routing: two tiny fp32 matmuls contract pooled over channels into
   PSUM [4,1]; ACT sigmoid with per-partition fc_b bias; the 4 gate
   values are broadcast to all 128 partitions with a diag(r) matmul.
 - mixing: DVE tensor_scalar + 3x affine_then_add accumulate
   W_eff = sum_e r_e * W_e into a float32r tile laid out as
   [i=128, iblk=2, pos=9, o=256] (conv lhsT-ready).
 - conv: 3x3/pad1 as 18 accumulating fp32r matmuls (2 iblk x 9 taps)
   per PSUM tile of 8 output rows (free dim 448); fp32r runs at bf16
   speed for free >= 256 with ~FP22 precision.
 - drain: DVE copy PSUM->SBUF staging, DMA out on the ACT HWDGE queue.
"""
import numpy as np
from contextlib import ExitStack

from concourse import bacc
import concourse.tile as tile
import concourse.mybir as mybir
from concourse.bass_utils import run_bass_kernel_spmd

F32 = mybir.dt.float32
F32R = mybir.dt.float32r

N_CORES = 8
B, C, H, W = 32, 256, 56, 56
E, O, K = 4, 256, 3
BC = B // N_CORES          # samples per core
HP = H + 2                 # padded spatial
POS = K * K
NIB = C // 128             # input-channel blocks
NOB = O // 128             # output-channel blocks
RB = 8                     # output rows per psum tile
NRB = H // RB              # row blocks per (sample, oblk)

_CACHED_NC = None


def build_nc():
    nc = bacc.Bacc(trn_type="TRN2")
    x = nc.dram_tensor("x", [BC, C, H, W], F32, kind="ExternalInput")
    w = nc.dram_tensor("w", [128, E, NIB, POS, O], F32, kind="ExternalInput")
    fcw = nc.dram_tensor("fcw", [128, NIB, E], F32, kind="ExternalInput")
    fcb = nc.dram_tensor("fcb", [E, 1], F32, kind="ExternalInput")
    eye = nc.dram_tensor("eye", [E, E], F32, kind="ExternalInput")
    out = nc.dram_tensor("out", [BC, O, H, W], F32, kind="ExternalOutput")

    with tile.TileContext(nc) as tc, ExitStack() as ctx:
        consts = ctx.enter_context(tc.tile_pool(name="consts", bufs=1))
        persist = ctx.enter_context(tc.tile_pool(name="persist", bufs=1))
        small = ctx.enter_context(tc.tile_pool(name="small", bufs=2))
        stage_p = ctx.enter_context(tc.tile_pool(name="stage", bufs=4))
        conv_ps = ctx.enter_context(tc.tile_pool(name="conv_ps", bufs=4, space="PSUM"))
        rout_ps = ctx.enter_context(tc.tile_pool(name="rout_ps", bufs=1, space="PSUM"))

        tw = consts.tile([128, E, NIB, POS, O], F32)
        nc.sync.dma_start(out=tw, in_=w[:, :, :, :, :])
        tfcw = consts.tile([128, NIB, E], F32)
        nc.sync.dma_start(out=tfcw, in_=fcw[:, :, :])
        tfcb = consts.tile([E, 1], F32)
        nc.sync.dma_start(out=tfcb, in_=fcb[:, :])
        teye = consts.tile([E, E], F32)
        nc.sync.dma_start(out=teye, in_=eye[:, :])
        ones = consts.tile([E, 128], F32)
        nc.vector.memset(ones, 1.0)

        zsrc = consts.tile([128, HP], F32)
        nc.vector.memset(zsrc, 0.0)

        # double-buffered padded input + mixed weights (distinct tags ->
        # persistent slots; borders zeroed once, only interiors rewritten)
        txs, weffs = [], []
        for s in range(2):
            t = persist.tile([128, NIB, HP, HP], F32R, tag=f"tx{s}", name=f"tx{s}")
            for ib in range(NIB):
                nc.vector.tensor_copy(out=t[:, ib, 0:1, :], in_=zsrc[:, :])
                nc.vector.tensor_copy(out=t[:, ib, HP - 1 : HP, :], in_=zsrc[:, :])
                nc.vector.tensor_copy(out=t[:, ib, :, 0:1], in_=zsrc[:, :])
                nc.vector.tensor_copy(out=t[:, ib, :, HP - 1 : HP], in_=zsrc[:, :])
            txs.append(t)
            weffs.append(persist.tile([128, NIB, POS, O], F32R, tag=f"weff{s}", name=f"weff{s}"))

        def prep(b):
            """Load x[b], pool, route, mix -> weff[b % 2]."""
            tx, weff = txs[b % 2], weffs[b % 2]
            for ib in range(NIB):
                nc.sync.dma_start(
                    out=tx[:, ib, 1 : H + 1, 1 : W + 1],
                    in_=x[b, ib * 128 : (ib + 1) * 128, :, :].bitcast(F32R),
                )
            pooled = small.tile([128, NIB], F32, tag="pooled")
            for ib in range(NIB):
                nc.scalar.activation(
                    out=tx[:, ib, :, :],
                    in_=tx[:, ib, :, :],
                    func=mybir.ActivationFunctionType.Copy,
                    accum_out=pooled[:, ib : ib + 1],
                )
            rps = rout_ps.tile([128, 8], F32, tag="rps")
            for ib in range(NIB):
                nc.tensor.matmul(
                    out=rps[0:E, 4:5],
                    lhsT=tfcw[:, ib, :],
                    rhs=pooled[:, ib : ib + 1],
                    start=(ib == 0),
                    stop=(ib == NIB - 1),
                )
            r_col = small.tile([E, 1], F32, tag="r_col")
            nc.scalar.activation(
                out=r_col[:, :],
                in_=rps[0:E, 4:5],
                func=mybir.ActivationFunctionType.Sigmoid,
                bias=tfcb[:, :],
            )
            tdiag = small.tile([E, E], F32, tag="tdiag")
            nc.vector.tensor_scalar(
                out=tdiag[:, :], in0=teye[:, :], scalar1=r_col[:, 0:1],
                scalar2=None, op0=mybir.AluOpType.mult,
            )
            nc.tensor.matmul(
                out=rps[:, 0:E], lhsT=ones[:, :], rhs=tdiag[:, :],
                start=True, stop=True,
            )
            r_sb = small.tile([128, E], F32, tag="r_sb")
            nc.vector.tensor_copy(out=r_sb[:, :], in_=rps[:, 0:E])
            for ib in range(NIB):
                nc.vector.tensor_scalar(
                    out=weff[:, ib, :, :],
                    in0=tw[:, 0, ib, :, :],
                    scalar1=r_sb[:, 0:1],
                    scalar2=None,
                    op0=mybir.AluOpType.mult,
                )
                for e in range(1, E):
                    nc.vector.affine_then_add(
                        out=weff[:, ib, :, :],
                        in0=tw[:, e, ib, :, :],
                        in1=weff[:, ib, :, :].bitcast(F32),
                        scale=r_sb[:, e : e + 1],
                        bias=0.0,
                    )

        def conv_half(b, ob):
            """One output-channel block of sample b's conv."""
            tx, weff = txs[b % 2], weffs[b % 2]
            for rb in range(NRB):
                r0 = rb * RB
                ct = conv_ps.tile([128, RB, W], F32, tag="ct")
                first = True
                for ib in range(NIB):
                    for ky in range(K):
                        for kx in range(K):
                            nc.tensor.matmul(
                                out=ct[:, :, :],
                                lhsT=weff[:, ib, ky * K + kx, ob * 128 : (ob + 1) * 128],
                                rhs=tx[:, ib, r0 + ky : r0 + ky + RB, kx : kx + W],
                                start=first,
                                stop=(ib == NIB - 1 and ky == K - 1 and kx == K - 1),
                            )
                            first = False
                st = stage_p.tile([128, RB, W], F32, tag="st")
                nc.vector.tensor_copy(out=st[:, :, :], in_=ct[:, :, :])
                nc.scalar.dma_start(
                    out=out[b, ob * 128 : (ob + 1) * 128, r0 : r0 + RB, :],
                    in_=st[:, :, :],
                )

        prep(0)
        for b in range(BC):
            conv_half(b, 0)
            if b + 1 < BC:
                prep(b + 1)
            conv_half(b, 1)

    nc.finalize()
    return nc


def _host_constants(weight, fc_w, fc_b):
    # w[i, e, ib, pos, o] = weight[e, o, ib*128+i, ky, kx]
    w = weight.reshape(E, O, NIB, 128, K, K).transpose(3, 0, 2, 4, 5, 1)
    w = np.ascontiguousarray(w.reshape(128, E, NIB, POS, O), dtype=np.float32)
    # fcw[i, ib, e] = fc_w[e, ib*128+i] / (H*W)  (folds the mean)
    fcw = np.ascontiguousarray(
        fc_w.reshape(E, NIB, 128).transpose(2, 1, 0), dtype=np.float32
    ) / float(H * W)
    fcb = np.ascontiguousarray(fc_b.reshape(E, 1), dtype=np.float32)
    eye = np.eye(E, dtype=np.float32)
    return w, fcw.astype(np.float32), fcb, eye


def make_in_maps(x, weight, fc_w, fc_b):
    w, fcw, fcb, eye = _host_constants(
        np.asarray(weight, dtype=np.float32),
        np.asarray(fc_w, dtype=np.float32),
        np.asarray(fc_b, dtype=np.float32),
    )
    x = np.ascontiguousarray(np.asarray(x, dtype=np.float32))
    return [
        {"x": x[c * BC : (c + 1) * BC], "w": w, "fcw": fcw, "fcb": fcb, "eye": eye}
        for c in range(N_CORES)
    ]


def get_nc():
    global _CACHED_NC
    if _CACHED_NC is None:
        _CACHED_NC = build_nc()
    return _CACHED_NC


def kernel(x, weight, fc_w, fc_b):
    in_maps = make_in_maps(x, weight, fc_w, fc_b)
    res = run_bass_kernel_spmd(get_nc(), in_maps, core_ids=list(range(N_CORES)))
    return np.concatenate([res.results[c]["out"] for c in range(N_CORES)], axis=0)


# revision 10
# speedup vs baseline: 377.7482x; 377.7482x over previous
"""CondConv2d Bass kernel for 8 Trainium2 NeuronCores.

Reference semantics (per sample b):
    pooled  = mean(x[b], axis=(H,W))                      # [C]
    r       = sigmoid(fc_w @ pooled + fc_b)               # [E]
    W_eff   = sum_e r[e] * weight[e]                      # [O, I, 3, 3]
    out[b]  = conv2d(x[b], W_eff, stride=1, pad=1)        # [O, H, W]

Sharding: data-parallel over batch B=32 across 8 cores (4 samples/core);
weights + routing FC replicated.

Per-core kernel design:
 - x image zero-padded to 58x58 in SBUF (one [128, 2, 58, 58] f32r tile
   per double-buffer slot); borders stay zero so a full-plane reduce
   equals the unpadded spatial sum.
 - pooling: ACT activation(Copy) with accum_out (in-place, scale=1);
   the 1/(H*W) mean factor is folded into the fc_w constant.
 - routing: two tiny fp32 matmuls contract pooled over channels into
   PSUM [4,1]; ACT sigmoid with per-partition fc_b bias; the 4 gate
   values are broadcast to all 128 partitions with a diag(r) matmul.
 - mixing: DVE tensor_scalar + 3x affine_then_add accumulate
   W_eff = sum_e r_e * W_e into a float32r tile laid out as
   [i=128, iblk=2, pos=9, o=256] (conv lhsT-ready).
 - conv: 3x3/pad1 as 18 accumulating fp32r matmuls (2 iblk x 9 taps)
   per PSUM tile of 8 output rows (free dim 448); fp32r runs at bf16
   speed for free >= 256 with ~FP22 precision.
 - drain: DVE copy PSUM->SBUF staging, DMA out on the ACT HWDGE queue.
"""
import numpy as np
from contextlib import ExitStack

from concourse import bacc
import concourse.tile as tile
import concourse.mybir as mybir
from concourse.bass_utils import run_bass_kernel_spmd

F32 = mybir.dt.float32
F32R = mybir.dt.float32r

N_CORES = 8
B, C, H, W = 32, 256, 56, 56
E, O, K = 4, 256, 3
BC = B // N_CORES          # samples per core
HP = H + 2                 # padded spatial
POS = K * K
NIB = C // 128             # input-channel blocks
NOB = O // 128             # output-channel blocks
RB = 8                     # output rows per psum tile
NRB = H // RB              # row blocks per (sample, oblk)

_CACHED_NC = None


def build_nc(repeat=1):
    nc = bacc.Bacc(trn_type="TRN2")
    x = nc.dram_tensor("x", [BC, C, H, W], F32, kind="ExternalInput")
    w = nc.dram_tensor("w", [128, E, NIB, POS, O], F32, kind="ExternalInput")
    fcw = nc.dram_tensor("fcw", [128, NIB, E], F32, kind="ExternalInput")
    fcb = nc.dram_tensor("fcb", [E, 1], F32, kind="ExternalInput")
    eye = nc.dram_tensor("eye", [E, E], F32, kind="ExternalInput")
    out = nc.dram_tensor("out", [BC, O, H, W], F32, kind="ExternalOutput")

    with tile.TileContext(nc) as tc, ExitStack() as ctx:
        consts = ctx.enter_context(tc.tile_pool(name="consts", bufs=1))
        persist = ctx.enter_context(tc.tile_pool(name="persist", bufs=1))
        small = ctx.enter_context(tc.tile_pool(name="small", bufs=2))
        stage_p = ctx.enter_context(tc.tile_pool(name="stage", bufs=6))
        conv_ps = ctx.enter_context(tc.tile_pool(name="conv_ps", bufs=6, space="PSUM"))
        rout_ps = ctx.enter_context(tc.tile_pool(name="rout_ps", bufs=1, space="PSUM"))

        tfcw = consts.tile([128, NIB, E], F32)
        nc.sync.dma_start(out=tfcw, in_=fcw[:, :, :])
        tfcb = consts.tile([E, 1], F32)
        nc.sync.dma_start(out=tfcb, in_=fcb[:, :])
        teye = consts.tile([E, E], F32)
        nc.sync.dma_start(out=teye, in_=eye[:, :])
        ones = consts.tile([E, 128], F32)
        nc.vector.memset(ones, 1.0)
        # per-expert weight tiles: DMA'd after the first sample's x so the
        # expert-e mixing pass can start as soon as its chunk lands
        tws = [
            consts.tile([128, NIB, POS, O], F32, name=f"tw{e}", tag=f"tw{e}")
            for e in range(E)
        ]

        zsrc = consts.tile([128, HP], F32)
        nc.vector.memset(zsrc, 0.0)

        # double-buffered padded input + mixed weights (distinct tags ->
        # persistent slots; borders zeroed once, only interiors rewritten)
        txs, weffs = [], []
        for s in range(2):
            t = persist.tile([128, NIB, HP, HP], F32R, tag=f"tx{s}", name=f"tx{s}")
            for ib in range(NIB):
                nc.vector.tensor_copy(out=t[:, ib, 0:1, :], in_=zsrc[:, :])
                nc.vector.tensor_copy(out=t[:, ib, HP - 1 : HP, :], in_=zsrc[:, :])
                nc.vector.tensor_copy(out=t[:, ib, :, 0:1], in_=zsrc[:, :])
                nc.vector.tensor_copy(out=t[:, ib, :, HP - 1 : HP], in_=zsrc[:, :])
            txs.append(t)
            weffs.append(persist.tile([128, NIB, POS, O], F32R, tag=f"weff{s}", name=f"weff{s}"))

        def prep_route(b):
            """Load x[b], pool, route -> r_sb."""
            tx = txs[b % 2]
            for ib in range(NIB):
                nc.sync.dma_start(
                    out=tx[:, ib, 1 : H + 1, 1 : W + 1],
                    in_=x[b, ib * 128 : (ib + 1) * 128, :, :].bitcast(F32R),
                )
            pooled = small.tile([128, NIB], F32, tag="pooled")
            for ib in range(NIB):
                nc.scalar.activation(
                    out=tx[:, ib, :, :],
                    in_=tx[:, ib, :, :],
                    func=mybir.ActivationFunctionType.Copy,
                    accum_out=pooled[:, ib : ib + 1],
                )
            rps = rout_ps.tile([128, 8], F32, tag="rps")
            for ib in range(NIB):
                nc.tensor.matmul(
                    out=rps[0:E, 4:5],
                    lhsT=tfcw[:, ib, :],
                    rhs=pooled[:, ib : ib + 1],
                    start=(ib == 0),
                    stop=(ib == NIB - 1),
                )
            r_col = small.tile([E, 1], F32, tag="r_col")
            nc.scalar.activation(
                out=r_col[:, :],
                in_=rps[0:E, 4:5],
                func=mybir.ActivationFunctionType.Sigmoid,
                bias=tfcb[:, :],
            )
            tdiag = small.tile([E, E], F32, tag="tdiag")
            nc.vector.tensor_scalar(
                out=tdiag[:, :], in0=teye[:, :], scalar1=r_col[:, 0:1],
                scalar2=None, op0=mybir.AluOpType.mult,
            )
            nc.tensor.matmul(
                out=rps[:, 0:E], lhsT=ones[:, :], rhs=tdiag[:, :],
                start=True, stop=True,
            )
            r_sb = small.tile([128, E], F32, tag="r_sb")
            nc.vector.tensor_copy(out=r_sb[:, :], in_=rps[:, 0:E])
            return r_sb

        def prep_mix(b, r_sb):
            """Mix weff[b % 2] = sum_e r_e * W_e (DVE)."""
            weff = weffs[b % 2]
            for ib in range(NIB):
                nc.vector.tensor_scalar(
                    out=weff[:, ib, :, :],
                    in0=tws[0][:, ib, :, :],
                    scalar1=r_sb[:, 0:1],
                    scalar2=None,
                    op0=mybir.AluOpType.mult,
                )
                for e in range(1, E):
                    nc.vector.affine_then_add(
                        out=weff[:, ib, :, :],
                        in0=tws[e][:, ib, :, :],
                        in1=weff[:, ib, :, :].bitcast(F32),
                        scale=r_sb[:, e : e + 1],
                        bias=0.0,
                    )

        def prep(b):
            prep_mix(b, prep_route(b))

        def conv_half(b, ob):
            """One output-channel block of sample b's conv."""
            tx, weff = txs[b % 2], weffs[b % 2]
            for rb in range(NRB):
                r0 = rb * RB
                ct = conv_ps.tile([128, RB, W], F32, tag="ct")
                first = True
                for ib in range(NIB):
                    for ky in range(K):
                        for kx in range(K):
                            nc.tensor.matmul(
                                out=ct[:, :, :],
                                lhsT=weff[:, ib, ky * K + kx, ob * 128 : (ob + 1) * 128],
                                rhs=tx[:, ib, r0 + ky : r0 + ky + RB, kx : kx + W],
                                start=first,
                                stop=(ib == NIB - 1 and ky == K - 1 and kx == K - 1),
                            )
                            first = False
                st = stage_p.tile([128, RB, W], F32, tag="st")
                nc.scalar.copy(out=st[:, :, :], in_=ct[:, :, :])
                nc.scalar.dma_start(
                    out=out[b, ob * 128 : (ob + 1) * 128, r0 : r0 + RB, :],
                    in_=st[:, :, :],
                )

        # prologue: first sample's load/route issues before the big weight
        # DMAs so pooling/routing overlap the weight transfer
        r0_sb = prep_route(0)
        for e in range(E):
            nc.sync.dma_start(out=tws[e], in_=w[:, e, :, :, :])
        prep_mix(0, r0_sb)
        for rep in range(repeat):
            for b in range(BC):
                conv_half(b, 0)
                if not (rep == repeat - 1 and b == BC - 1):
                    prep((b + 1) % BC)
                conv_half(b, 1)

    nc.finalize()
    return nc


def _host_constants(weight, fc_w, fc_b):
    # w[i, e, ib, pos, o] = weight[e, o, ib*128+i, ky, kx]
    w = weight.reshape(E, O, NIB, 128, K, K).transpose(3, 0, 2, 4, 5, 1)
    w = np.ascontiguousarray(w.reshape(128, E, NIB, POS, O), dtype=np.float32)
    # fcw[i, ib, e] = fc_w[e, ib*128+i] / (H*W)  (folds the mean)
    fcw = np.ascontiguousarray(
        fc_w.reshape(E, NIB, 128).transpose(2, 1, 0), dtype=np.float32
    ) / float(H * W)
    fcb = np.ascontiguousarray(fc_b.reshape(E, 1), dtype=np.float32)
    eye = np.eye(E, dtype=np.float32)
    return w, fcw.astype(np.float32), fcb, eye


def make_in_maps(x, weight, fc_w, fc_b):
    w, fcw, fcb, eye = _host_constants(
        np.asarray(weight, dtype=np.float32),
        np.asarray(fc_w, dtype=np.float32),
        np.asarray(fc_b, dtype=np.float32),
    )
    x = np.ascontiguousarray(np.asarray(x, dtype=np.float32))
    return [
        {"x": x[c * BC : (c + 1) * BC], "w": w, "fcw": fcw, "fcb": fcb, "eye": eye}
        for c in range(N_CORES)
    ]


def get_nc():
    global _CACHED_NC
    if _CACHED_NC is None:
        _CACHED_NC = build_nc()
    return _CACHED_NC


def kernel(x, weight, fc_w, fc_b):
    in_maps = make_in_maps(x, weight, fc_w, fc_b)
    res = run_bass_kernel_spmd(get_nc(), in_maps, core_ids=list(range(N_CORES)))
    return np.concatenate([res.results[c]["out"] for c in range(N_CORES)], axis=0)


# revision 17
# speedup vs baseline: 404.2431x; 1.0701x over previous
"""CondConv2d Bass kernel for 8 Trainium2 NeuronCores.

Reference semantics (per sample b):
    pooled  = mean(x[b], axis=(H,W))                      # [C]
    r       = sigmoid(fc_w @ pooled + fc_b)               # [E]
    W_eff   = sum_e r[e] * weight[e]                      # [O, I, 3, 3]
    out[b]  = conv2d(x[b], W_eff, stride=1, pad=1)        # [O, H, W]

Sharding: data-parallel over batch B=32 across 8 cores (4 samples/core);
weights + routing FC replicated.

Per-core kernel design:
 - x image zero-padded to 58x58 in SBUF (one [128, 2, 58, 58] f32r tile
   per double-buffer slot); borders stay zero so a full-plane reduce
   equals the unpadded spatial sum.
 - pooling: ACT activation(Copy) with accum_out (in-place, scale=1);
   the 1/(H*W) mean factor is folded into the fc_w constant.
 - routing: two tiny fp32 matmuls contract pooled over channels into
   PSUM [4,1]; ACT sigmoid with per-partition fc_b bias; the 4 gate
   values are broadcast to all 128 partitions with a diag(r) matmul.
 - mixing: DVE tensor_scalar + 3x affine_then_add accumulate
   W_eff = sum_e r_e * W_e into a float32r tile laid out as
   [i=128, iblk=2, pos=9, o=256] (conv lhsT-ready).
 - conv: 3x3/pad1 as 18 accumulating fp32r matmuls (2 iblk x 9 taps)
   per PSUM tile of 8 output rows (free dim 448); fp32r runs at bf16
   speed for free >= 256 with ~FP22 precision.
 - drain: DVE copy PSUM->SBUF staging, DMA out on the ACT HWDGE queue.
"""
import numpy as np
from contextlib import ExitStack

from concourse import bacc
import concourse.tile as tile
import concourse.mybir as mybir
from concourse.bass_utils import run_bass_kernel_spmd

F32 = mybir.dt.float32
F32R = mybir.dt.float32r

N_CORES = 8
B, C, H, W = 32, 256, 56, 56
E, O, K = 4, 256, 3
BC = B // N_CORES          # samples per core
HP = H + 2                 # padded spatial
POS = K * K
NIB = C // 128             # input-channel blocks
NOB = O // 128             # output-channel blocks
RB = 8                     # output rows per psum tile
NRB = H // RB              # row blocks per (sample, oblk)

_CACHED_NC = None


def build_nc(repeat=1, ws=True):
    nc = bacc.Bacc(trn_type="TRN2")
    x = nc.dram_tensor("x", [BC, C, H, W], F32, kind="ExternalInput")
    w = nc.dram_tensor("w", [128, E, NIB, POS, O], F32, kind="ExternalInput")
    fcw = nc.dram_tensor("fcw", [128, NIB, E], F32, kind="ExternalInput")
    fcb = nc.dram_tensor("fcb", [E, 1], F32, kind="ExternalInput")
    eye = nc.dram_tensor("eye", [E, E], F32, kind="ExternalInput")
    out = nc.dram_tensor("out", [BC, O, H, W], F32, kind="ExternalOutput")

    with tile.TileContext(nc) as tc, ExitStack() as ctx:
        consts = ctx.enter_context(tc.tile_pool(name="consts", bufs=1))
        persist = ctx.enter_context(tc.tile_pool(name="persist", bufs=1))
        small = ctx.enter_context(tc.tile_pool(name="small", bufs=2))
        stage_p = ctx.enter_context(tc.tile_pool(name="stage", bufs=6))
        conv_ps = ctx.enter_context(
            tc.tile_pool(name="conv_ps", bufs=1 if ws else 6, space="PSUM")
        )
        rout_ps = ctx.enter_context(tc.tile_pool(name="rout_ps", bufs=1, space="PSUM"))

        tfcw = consts.tile([128, NIB, E], F32)
        nc.sync.dma_start(out=tfcw, in_=fcw[:, :, :])
        tfcb = consts.tile([E, 1], F32)
        nc.sync.dma_start(out=tfcb, in_=fcb[:, :])
        teye = consts.tile([E, E], F32)
        nc.sync.dma_start(out=teye, in_=eye[:, :])
        ones = consts.tile([E, 128], F32)
        nc.vector.memset(ones, 1.0)
        # per-expert weight tiles: DMA'd after the first sample's x so the
        # expert-e mixing pass can start as soon as its chunk lands
        tws = [
            consts.tile([128, NIB, POS, O], F32, name=f"tw{e}", tag=f"tw{e}")
            for e in range(E)
        ]

        zsrc = consts.tile([128, HP], F32)
        nc.vector.memset(zsrc, 0.0)

        # double-buffered padded input + mixed weights (distinct tags ->
        # persistent slots; borders zeroed once, only interiors rewritten)
        txs, weffs = [], []
        for s in range(2):
            t = persist.tile([128, NIB, HP, HP], F32R, tag=f"tx{s}", name=f"tx{s}")
            for ib in range(NIB):
                nc.vector.tensor_copy(out=t[:, ib, 0:1, :], in_=zsrc[:, :])
                nc.vector.tensor_copy(out=t[:, ib, HP - 1 : HP, :], in_=zsrc[:, :])
                nc.vector.tensor_copy(out=t[:, ib, :, 0:1], in_=zsrc[:, :])
                nc.vector.tensor_copy(out=t[:, ib, :, HP - 1 : HP], in_=zsrc[:, :])
            txs.append(t)
            weffs.append([
                persist.tile([128, POS, O], F32R, tag=f"weff{s}_{ib}",
                             name=f"weff{s}_{ib}")
                for ib in range(NIB)
            ])

        def prep_route(b):
            """Load x[b], pool, route -> r_sb."""
            tx = txs[b % 2]
            for ib in range(NIB):
                nc.sync.dma_start(
                    out=tx[:, ib, 1 : H + 1, 1 : W + 1],
                    in_=x[b, ib * 128 : (ib + 1) * 128, :, :].bitcast(F32R),
                )
            pooled = small.tile([128, NIB], F32, tag="pooled")
            for ib in range(NIB):
                nc.scalar.activation(
                    out=tx[:, ib, :, :],
                    in_=tx[:, ib, :, :],
                    func=mybir.ActivationFunctionType.Copy,
                    accum_out=pooled[:, ib : ib + 1],
                )
            rps = rout_ps.tile([128, 8], F32, tag="rps")
            for ib in range(NIB):
                nc.tensor.matmul(
                    out=rps[0:E, 4:5],
                    lhsT=tfcw[:, ib, :],
                    rhs=pooled[:, ib : ib + 1],
                    start=(ib == 0),
                    stop=(ib == NIB - 1),
                )
            r_col = small.tile([E, 1], F32, tag="r_col")
            nc.scalar.activation(
                out=r_col[:, :],
                in_=rps[0:E, 4:5],
                func=mybir.ActivationFunctionType.Sigmoid,
                bias=tfcb[:, :],
            )
            tdiag = small.tile([E, E], F32, tag="tdiag")
            nc.vector.tensor_scalar(
                out=tdiag[:, :], in0=teye[:, :], scalar1=r_col[:, 0:1],
                scalar2=None, op0=mybir.AluOpType.mult,
            )
            nc.tensor.matmul(
                out=rps[:, 0:E], lhsT=ones[:, :], rhs=tdiag[:, :],
                start=True, stop=True,
            )
            r_sb = small.tile([128, E], F32, tag="r_sb")
            nc.vector.tensor_copy(out=r_sb[:, :], in_=rps[:, 0:E])
            return r_sb

        def prep_mix(b, r_sb):
            """Mix weff[b % 2][ib] = sum_e r_e * W_e (DVE)."""
            weff = weffs[b % 2]
            for ib in range(NIB):
                nc.vector.tensor_scalar(
                    out=weff[ib][:, :, :],
                    in0=tws[0][:, ib, :, :],
                    scalar1=r_sb[:, 0:1],
                    scalar2=None,
                    op0=mybir.AluOpType.mult,
                )
                for e in range(1, E):
                    nc.vector.affine_then_add(
                        out=weff[ib][:, :, :],
                        in0=tws[e][:, ib, :, :],
                        in1=weff[ib][:, :, :].bitcast(F32),
                        scale=r_sb[:, e : e + 1],
                        bias=0.0,
                    )

        def prep(b):
            prep_mix(b, prep_route(b))

        def conv_half_ws(b, ob):
            """Weight-stationary variant: 4+3 PSUM row-block groups per
            oblk; each weight tile feeds all blocks of a group back-to-back
            so consecutive matmuls reuse the loaded weights."""
            tx, weff = txs[b % 2], weffs[b % 2]
            for g0, gn in ((0, 4), (4, 3)):
                cts = [
                    conv_ps.tile([128, RB, W], F32, tag=f"ct{g0 + i}",
                                 name=f"ct{b}_{ob}_{g0 + i}")
                    for i in range(gn)
                ]
                for k, (ib, ky, kx) in enumerate(
                    (ib, ky, kx) for ib in range(NIB) for ky in range(K) for kx in range(K)
                ):
                    lhs = weff[ib][:, ky * K + kx, ob * 128 : (ob + 1) * 128]
                    for i in range(gn):
                        r0 = (g0 + i) * RB
                        nc.tensor.matmul(
                            out=cts[i][:, :, :],
                            lhsT=lhs,
                            rhs=tx[:, ib, r0 + ky : r0 + ky + RB, kx : kx + W],
                            start=(k == 0),
                            stop=(k == NIB * POS - 1),
                        )
                for i in range(gn):
                    r0 = (g0 + i) * RB
                    st = stage_p.tile([128, RB, W], F32, tag="st")
                    nc.scalar.copy(out=st[:, :, :], in_=cts[i][:, :, :])
                    nc.scalar.dma_start(
                        out=out[b, ob * 128 : (ob + 1) * 128, r0 : r0 + RB, :],
                        in_=st[:, :, :],
                    )

        def conv_half(b, ob):
            """One output-channel block of sample b's conv."""
            if ws:
                return conv_half_ws(b, ob)
            tx, weff = txs[b % 2], weffs[b % 2]
            for rb in range(NRB):
                r0 = rb * RB
                ct = conv_ps.tile([128, RB, W], F32, tag="ct")
                first = True
                for ib in range(NIB):
                    for ky in range(K):
                        for kx in range(K):
                            nc.tensor.matmul(
                                out=ct[:, :, :],
                                lhsT=weff[ib][:, ky * K + kx, ob * 128 : (ob + 1) * 128],
                                rhs=tx[:, ib, r0 + ky : r0 + ky + RB, kx : kx + W],
                                start=first,
                                stop=(ib == NIB - 1 and ky == K - 1 and kx == K - 1),
                            )
                            first = False
                st = stage_p.tile([128, RB, W], F32, tag="st")
                nc.scalar.copy(out=st[:, :, :], in_=ct[:, :, :])
                nc.scalar.dma_start(
                    out=out[b, ob * 128 : (ob + 1) * 128, r0 : r0 + RB, :],
                    in_=st[:, :, :],
                )

        # prologue: first sample's load/route issues before the big weight
        # DMAs so pooling/routing overlap the weight transfer
        r0_sb = prep_route(0)
        for ib in range(NIB):
            for e in range(E):
                nc.sync.dma_start(
                    out=tws[e][:, ib, :, :], in_=w[:, e, ib, :, :]
                )
        prep_mix(0, r0_sb)
        for rep in range(repeat):
            for b in range(BC):
                conv_half(b, 0)
                if not (rep == repeat - 1 and b == BC - 1):
                    prep((b + 1) % BC)
                conv_half(b, 1)

    nc.finalize()
    return nc


def _host_constants(weight, fc_w, fc_b):
    # w[i, e, ib, pos, o] = weight[e, o, ib*128+i, ky, kx]
    w = weight.reshape(E, O, NIB, 128, K, K).transpose(3, 0, 2, 4, 5, 1)
    w = np.ascontiguousarray(w.reshape(128, E, NIB, POS, O), dtype=np.float32)
    # fcw[i, ib, e] = fc_w[e, ib*128+i] / (H*W)  (folds the mean)
    fcw = np.ascontiguousarray(
        fc_w.reshape(E, NIB, 128).transpose(2, 1, 0), dtype=np.float32
    ) / float(H * W)
    fcb = np.ascontiguousarray(fc_b.reshape(E, 1), dtype=np.float32)
    eye = np.eye(E, dtype=np.float32)
    return w, fcw.astype(np.float32), fcb, eye


def make_in_maps(x, weight, fc_w, fc_b):
    w, fcw, fcb, eye = _host_constants(
        np.asarray(weight, dtype=np.float32),
        np.asarray(fc_w, dtype=np.float32),
        np.asarray(fc_b, dtype=np.float32),
    )
    x = np.ascontiguousarray(np.asarray(x, dtype=np.float32))
    return [
        {"x": x[c * BC : (c + 1) * BC], "w": w, "fcw": fcw, "fcb": fcb, "eye": eye}
        for c in range(N_CORES)
    ]


def get_nc():
    global _CACHED_NC
    if _CACHED_NC is None:
        _CACHED_NC = build_nc()
    return _CACHED_NC


def kernel(x, weight, fc_w, fc_b):
    in_maps = make_in_maps(x, weight, fc_w, fc_b)
    res = run_bass_kernel_spmd(get_nc(), in_maps, core_ids=list(range(N_CORES)))
    return np.concatenate([res.results[c]["out"] for c in range(N_CORES)], axis=0)


# revision 20
# speedup vs baseline: 443.1223x; 1.0962x over previous
"""CondConv2d Bass kernel for 8 Trainium2 NeuronCores.

Reference semantics (per sample b):
    pooled  = mean(x[b], axis=(H,W))                      # [C]
    r       = sigmoid(fc_w @ pooled + fc_b)               # [E]
    W_eff   = sum_e r[e] * weight[e]                      # [O, I, 3, 3]
    out[b]  = conv2d(x[b], W_eff, stride=1, pad=1)        # [O, H, W]

Sharding: data-parallel over batch B=32 across 8 cores (4 samples/core);
weights + routing FC replicated.

Per-core kernel design:
 - x image zero-padded to 58x58 in SBUF (one [128, 2, 58, 58] f32r tile
   per double-buffer slot); borders stay zero so a full-plane reduce
   equals the unpadded spatial sum.
 - pooling: ACT activation(Copy) with accum_out (in-place, scale=1);
   the 1/(H*W) mean factor is folded into the fc_w constant.
 - routing: two tiny fp32 matmuls contract pooled over channels into
   PSUM [4,1]; ACT sigmoid with per-partition fc_b bias; the 4 gate
   values are broadcast to all 128 partitions with a diag(r) matmul.
 - mixing: DVE tensor_scalar + 3x affine_then_add accumulate
   W_eff = sum_e r_e * W_e into per-iblock float32r tiles laid out as
   [i=128, pos=9, o=256] (conv lhsT-ready).
 - conv: 3x3/pad1 as 18 accumulating fp32r matmuls (2 iblk x 9 taps)
   per PSUM tile of 8 output rows (free dim 448); fp32r runs at bf16
   speed for free >= 256 with ~FP22 precision. Weight-stationary 4+3
   row-block groups let consecutive matmuls reuse the loaded weights.
 - drain: ACT copy PSUM->SBUF staging, DMA out on the ACT HWDGE queue.
 - pipelining: double-buffered tx/weff; sample b+1's load/route/mix is
   emitted between sample b's two output-channel halves so DVE/ACT prep
   overlaps PE conv; weights stream in per-(iblk, expert) chunks behind
   the first sample's x so mixing starts as chunks land.
"""
import numpy as np
from contextlib import ExitStack

from concourse import bacc
import concourse.tile as tile
import concourse.mybir as mybir
from concourse.bass_utils import run_bass_kernel_spmd

F32 = mybir.dt.float32
F32R = mybir.dt.float32r

N_CORES = 8
B, C, H, W = 32, 256, 56, 56
E, O, K = 4, 256, 3
BC = B // N_CORES          # samples per core
HP = H + 2                 # padded spatial
POS = K * K
NIB = C // 128             # input-channel blocks
NOB = O // 128             # output-channel blocks
RB = 8                     # output rows per psum tile
NRB = H // RB              # row blocks per (sample, oblk)

_CACHED_NC = None


def build_nc(repeat=1, ws=True):
    nc = bacc.Bacc(trn_type="TRN2")
    x = nc.dram_tensor("x", [BC, C, H, W], F32, kind="ExternalInput")
    w = nc.dram_tensor("w", [128, E, NIB, POS, O], F32, kind="ExternalInput")
    fcw = nc.dram_tensor("fcw", [128, NIB, E], F32, kind="ExternalInput")
    fcb = nc.dram_tensor("fcb", [E, 1], F32, kind="ExternalInput")
    eye = nc.dram_tensor("eye", [E, E], F32, kind="ExternalInput")
    out = nc.dram_tensor("out", [BC, O, H, W], F32, kind="ExternalOutput")

    with tile.TileContext(nc) as tc, ExitStack() as ctx:
        consts = ctx.enter_context(tc.tile_pool(name="consts", bufs=1))
        persist = ctx.enter_context(tc.tile_pool(name="persist", bufs=1))
        small = ctx.enter_context(tc.tile_pool(name="small", bufs=2))
        stage_p = ctx.enter_context(tc.tile_pool(name="stage", bufs=6))
        conv_ps = ctx.enter_context(
            tc.tile_pool(name="conv_ps", bufs=1 if ws else 6, space="PSUM")
        )
        rout_ps = ctx.enter_context(tc.tile_pool(name="rout_ps", bufs=1, space="PSUM"))

        tfcw = consts.tile([128, NIB, E], F32)
        nc.sync.dma_start(out=tfcw, in_=fcw[:, :, :])
        tfcb = consts.tile([E, 1], F32)
        nc.sync.dma_start(out=tfcb, in_=fcb[:, :])
        teye = consts.tile([E, E], F32)
        nc.sync.dma_start(out=teye, in_=eye[:, :])
        ones = consts.tile([E, 128], F32)
        nc.vector.memset(ones, 1.0)
        # per-expert weight tiles: DMA'd after the first sample's x so the
        # expert-e mixing pass can start as soon as its chunk lands
        tws = [
            consts.tile([128, NIB, POS, O], F32, name=f"tw{e}", tag=f"tw{e}")
            for e in range(E)
        ]

        zsrc = consts.tile([128, HP], F32)
        nc.vector.memset(zsrc, 0.0)

        # double-buffered padded input + mixed weights (distinct tags ->
        # persistent slots; borders zeroed once, only interiors rewritten)
        def zero_borders(t):
            for ib in range(NIB):
                nc.vector.tensor_copy(out=t[:, ib, 0:1, :], in_=zsrc[:, :])
                nc.vector.tensor_copy(out=t[:, ib, HP - 1 : HP, :], in_=zsrc[:, :])
                nc.vector.tensor_copy(out=t[:, ib, :, 0:1], in_=zsrc[:, :])
                nc.vector.tensor_copy(out=t[:, ib, :, HP - 1 : HP], in_=zsrc[:, :])

        txs, weffs = [], []
        for s in range(2):
            t = persist.tile([128, NIB, HP, HP], F32R, tag=f"tx{s}", name=f"tx{s}")
            txs.append(t)
            weffs.append([
                persist.tile([128, POS, O], F32R, tag=f"weff{s}_{ib}",
                             name=f"weff{s}_{ib}")
                for ib in range(NIB)
            ])

        def prep_route(b):
            """Load x[b], pool, route -> r_sb."""
            tx = txs[b % 2]
            for ib in range(NIB):
                nc.sync.dma_start(
                    out=tx[:, ib, 1 : H + 1, 1 : W + 1],
                    in_=x[b, ib * 128 : (ib + 1) * 128, :, :].bitcast(F32R),
                )
            pooled = small.tile([128, NIB], F32, tag="pooled")
            for ib in range(NIB):
                nc.scalar.activation(
                    out=tx[:, ib, :, :],
                    in_=tx[:, ib, :, :],
                    func=mybir.ActivationFunctionType.Copy,
                    accum_out=pooled[:, ib : ib + 1],
                )
            rps = rout_ps.tile([128, 8], F32, tag="rps")
            for ib in range(NIB):
                nc.tensor.matmul(
                    out=rps[0:E, 4:5],
                    lhsT=tfcw[:, ib, :],
                    rhs=pooled[:, ib : ib + 1],
                    start=(ib == 0),
                    stop=(ib == NIB - 1),
                )
            r_col = small.tile([E, 1], F32, tag="r_col")
            nc.scalar.activation(
                out=r_col[:, :],
                in_=rps[0:E, 4:5],
                func=mybir.ActivationFunctionType.Sigmoid,
                bias=tfcb[:, :],
            )
            tdiag = small.tile([E, E], F32, tag="tdiag")
            nc.vector.tensor_scalar(
                out=tdiag[:, :], in0=teye[:, :], scalar1=r_col[:, 0:1],
                scalar2=None, op0=mybir.AluOpType.mult,
            )
            nc.tensor.matmul(
                out=rps[:, 0:E], lhsT=ones[:, :], rhs=tdiag[:, :],
                start=True, stop=True,
            )
            r_sb = small.tile([128, E], F32, tag="r_sb")
            nc.vector.tensor_copy(out=r_sb[:, :], in_=rps[:, 0:E])
            return r_sb

        def prep_mix(b, r_sb):
            """Mix weff[b % 2][ib] = sum_e r_e * W_e (DVE)."""
            weff = weffs[b % 2]
            for ib in range(NIB):
                nc.vector.tensor_scalar(
                    out=weff[ib][:, :, :],
                    in0=tws[0][:, ib, :, :],
                    scalar1=r_sb[:, 0:1],
                    scalar2=None,
                    op0=mybir.AluOpType.mult,
                )
                for e in range(1, E):
                    nc.vector.affine_then_add(
                        out=weff[ib][:, :, :],
                        in0=tws[e][:, ib, :, :],
                        in1=weff[ib][:, :, :].bitcast(F32),
                        scale=r_sb[:, e : e + 1],
                        bias=0.0,
                    )

        def prep(b):
            prep_mix(b, prep_route(b))

        def conv_half_ws(b, ob):
            """Weight-stationary variant: 4+3 PSUM row-block groups per
            oblk; each weight tile feeds all blocks of a group back-to-back
            so consecutive matmuls reuse the loaded weights."""
            tx, weff = txs[b % 2], weffs[b % 2]
            for g0, gn in ((0, 4), (4, 3)):
                cts = [
                    conv_ps.tile([128, RB, W], F32, tag=f"ct{g0 + i}",
                                 name=f"ct{b}_{ob}_{g0 + i}")
                    for i in range(gn)
                ]
                for k, (ib, ky, kx) in enumerate(
                    (ib, ky, kx) for ib in range(NIB) for ky in range(K) for kx in range(K)
                ):
                    lhs = weff[ib][:, ky * K + kx, ob * 128 : (ob + 1) * 128]
                    for i in range(gn):
                        r0 = (g0 + i) * RB
                        nc.tensor.matmul(
                            out=cts[i][:, :, :],
                            lhsT=lhs,
                            rhs=tx[:, ib, r0 + ky : r0 + ky + RB, kx : kx + W],
                            start=(k == 0),
                            stop=(k == NIB * POS - 1),
                        )
                for i in range(gn):
                    r0 = (g0 + i) * RB
                    st = stage_p.tile([128, RB, W], F32, tag="st")
                    nc.scalar.copy(out=st[:, :, :], in_=cts[i][:, :, :])
                    nc.scalar.dma_start(
                        out=out[b, ob * 128 : (ob + 1) * 128, r0 : r0 + RB, :],
                        in_=st[:, :, :],
                    )

        def conv_half(b, ob):
            """One output-channel block of sample b's conv."""
            if ws:
                return conv_half_ws(b, ob)
            tx, weff = txs[b % 2], weffs[b % 2]
            for rb in range(NRB):
                r0 = rb * RB
                ct = conv_ps.tile([128, RB, W], F32, tag="ct")
                first = True
                for ib in range(NIB):
                    for ky in range(K):
                        for kx in range(K):
                            nc.tensor.matmul(
                                out=ct[:, :, :],
                                lhsT=weff[ib][:, ky * K + kx, ob * 128 : (ob + 1) * 128],
                                rhs=tx[:, ib, r0 + ky : r0 + ky + RB, kx : kx + W],
                                start=first,
                                stop=(ib == NIB - 1 and ky == K - 1 and kx == K - 1),
                            )
                            first = False
                st = stage_p.tile([128, RB, W], F32, tag="st")
                nc.scalar.copy(out=st[:, :, :], in_=ct[:, :, :])
                nc.scalar.dma_start(
                    out=out[b, ob * 128 : (ob + 1) * 128, r0 : r0 + RB, :],
                    in_=st[:, :, :],
                )

        # prologue: first sample's load/route issues before the big weight
        # DMAs so pooling/routing overlap the weight transfer
        zero_borders(txs[0])
        r0_sb = prep_route(0)
        for ib in range(NIB):
            for e in range(E):
                nc.sync.dma_start(
                    out=tws[e][:, ib, :, :], in_=w[:, e, ib, :, :]
                )
        prep_mix(0, r0_sb)
        zero_borders(txs[1])
        for rep in range(repeat):
            for b in range(BC):
                conv_half(b, 0)
                if not (rep == repeat - 1 and b == BC - 1):
                    prep((b + 1) % BC)
                conv_half(b, 1)

    nc.finalize()
    return nc


def _host_constants(weight, fc_w, fc_b):
    # w[i, e, ib, pos, o] = weight[e, o, ib*128+i, ky, kx]
    w = weight.reshape(E, O, NIB, 128, K, K).transpose(3, 0, 2, 4, 5, 1)
    w = np.ascontiguousarray(w.reshape(128, E, NIB, POS, O), dtype=np.float32)
    # fcw[i, ib, e] = fc_w[e, ib*128+i] / (H*W)  (folds the mean)
    fcw = np.ascontiguousarray(
        fc_w.reshape(E, NIB, 128).transpose(2, 1, 0), dtype=np.float32
    ) / float(H * W)
    fcb = np.ascontiguousarray(fc_b.reshape(E, 1), dtype=np.float32)
    eye = np.eye(E, dtype=np.float32)
    return w, fcw.astype(np.float32), fcb, eye


def make_in_maps(x, weight, fc_w, fc_b):
    w, fcw, fcb, eye = _host_constants(
        np.asarray(weight, dtype=np.float32),
        np.asarray(fc_w, dtype=np.float32),
        np.asarray(fc_b, dtype=np.float32),
    )
    x = np.ascontiguousarray(np.asarray(x, dtype=np.float32))
    return [
        {"x": x[c * BC : (c + 1) * BC], "w": w, "fcw": fcw, "fcb": fcb, "eye": eye}
        for c in range(N_CORES)
    ]


def get_nc():
    global _CACHED_NC
    if _CACHED_NC is None:
        _CACHED_NC = build_nc()
    return _CACHED_NC


def kernel(x, weight, fc_w, fc_b):
    in_maps = make_in_maps(x, weight, fc_w, fc_b)
    res = run_bass_kernel_spmd(get_nc(), in_maps, core_ids=list(range(N_CORES)))
    return np.concatenate([res.results[c]["out"] for c in range(N_CORES)], axis=0)
